# revision 1
# baseline (speedup 1.0000x reference)
"""Llama attention (B=2, S=2048, H=2048, NH=32, NKV=8, D=64) on 8 trn2 cores.

Sharding: tensor-parallel over heads. Core c owns q-heads [4c, 4c+4) and
kv-head c (GQA groups stay aligned). Each core computes its partial
out_c = attn_c @ wo[:, 256c:256c+256].T over the full batch/sequence in
bf16; the host sums the 8 partials in f32.

Device layout notes:
  - projections computed "feature-major": q/k [d, tok] via stationary
    weight tiles streaming xT; v is PE-transposed to token-major and
    augmented with a ones column so the AV matmul also produces softmax
    denominators.
  - q heads are stored pair-stacked ([head 2p | head 2p+1] on partitions)
    and k is duplicated into both partition halves, so the two scoresT
    matmuls of a pair run concurrently on disjoint PE row-groups (K=64
    each). One 3D-AP exp covers both heads.
  - causal: partial-width matmuls + a triangular mask multiply on the
    diagonal 128x128 block of each head.
  - softmax normalization: reciprocal_approx_fast on the sums row,
    broadcast across partitions with a K=1 fp16 matmul, folded into the
    psum->sbuf copy. The output projection is interleaved per q-chunk to
    keep the PE warm while ACT computes exponentials.
"""

import sys

if "/opt/trn_rl_repo" not in sys.path:
    sys.path.insert(0, "/opt/trn_rl_repo")

import numpy as np
import ml_dtypes

import concourse.bass as bass
import concourse.mybir as mybir
import concourse.tile as tile
from concourse import bacc
from concourse.masks import make_identity

bf16 = mybir.dt.bfloat16
f16 = mybir.dt.float16
f32 = mybir.dt.float32
AF = mybir.ActivationFunctionType

B = 2
D = 64
QH = 4                      # q heads per core
SCALE = D ** -0.5


def _segments(lo, hi, step=512):
    """Split [lo, hi) at multiples of `step` (matmul one-psum-bank limit)."""
    out = []
    while lo < hi:
        nxt = min(hi, (lo // step + 1) * step)
        out.append((lo, nxt))
        lo = nxt
    return out


def build(Sb=2048, H=2048, NGW=1024, QCW=512):
    """Sb: tokens per batch; H: model dim; NGW: stage-1 token group width;
    QCW: per-head q-chunk width in stage 2 (<= 512)."""
    assert QCW <= 512
    ST = B * Sb             # total tokens
    KT = H // 128           # contraction tiles for projections
    DQ = QH * D             # 256
    NP = QH // 2            # head pairs per core

    nc = bacc.Bacc(trn_type="TRN2")
    xT_d = nc.dram_tensor("xT", [H, ST], bf16, kind="ExternalInput")
    wqkv_d = nc.dram_tensor("wqkvT", [H, DQ + 2 * D], bf16, kind="ExternalInput")
    wo_d = nc.dram_tensor("woT", [DQ, H], bf16, kind="ExternalInput")
    cos2_d = nc.dram_tensor("cos2", [128, ST], bf16, kind="ExternalInput")
    sinadj_d = nc.dram_tensor("sinadj", [128, ST], bf16, kind="ExternalInput")
    out_d = nc.dram_tensor("out", [ST, H], bf16, kind="ExternalOutput")

    with tile.TileContext(nc) as tc:
        with (
            tc.tile_pool(name="consts", bufs=1) as consts,
            tc.tile_pool(name="resident", bufs=1) as res,
            tc.tile_pool(name="xpool", bufs=3) as xpool,
            tc.tile_pool(name="scratch", bufs=3) as scratch,
            tc.tile_pool(name="etp", bufs=6) as etp,
            tc.tile_pool(name="npool", bufs=3) as npool,
            tc.tile_pool(name="obuf", bufs=3) as obuf,
        ):
            ident = consts.tile([D, D], bf16, name="ident")
            make_identity(nc, ident)
            ones64 = consts.tile([1, D], f16, name="ones64")
            nc.vector.memset(ones64[:], 1.0)
            trimask = consts.tile([128, 128], bf16, name="trimask")
            nc.vector.memset(trimask[:], 1.0)
            nc.gpsimd.affine_select(
                out=trimask[:], in_=trimask[:],
                compare_op=mybir.AluOpType.is_ge, fill=0.0,
                base=0, pattern=[[1, 128]], channel_multiplier=-1,
            )

            cos2 = res.tile([128, ST], bf16, name="cos2")
            nc.sync.dma_start(cos2[:], cos2_d[:])
            sinadj = res.tile([128, ST], bf16, name="sinadj")
            nc.sync.dma_start(sinadj[:], sinadj_d[:])

            wo_t = []
            for ki in range(DQ // 128):
                w = res.tile([128, H], bf16, name=f"wo{ki}")
                nc.sync.dma_start(w[:], wo_d[ki * 128:(ki + 1) * 128, :])
                wo_t.append(w)

            wqkv_t = []
            for kt in range(KT):
                w = res.tile([128, DQ + 2 * D], bf16, name=f"wqkv{kt}")
                nc.sync.dma_start(w[:], wqkv_d[kt * 128:(kt + 1) * 128, :])
                wqkv_t.append(w)

            # pair-stacked q, duplicated k, pair-stacked attention output
            q2 = [res.tile([128, ST], bf16, name=f"q2_{p}") for p in range(NP)]
            k2 = res.tile([128, ST], bf16, name="k2")
            attnT = [res.tile([128, ST], bf16, name=f"attnT{p}") for p in range(NP)]
            vaug = [res.tile([128, D + 2], bf16, name=f"vaug{i}")
                    for i in range(ST // 128)]

            NM = NP + 1  # q pair m-tiles + 1 kv tile

            # ---------------- stage 1: QKV projection + RoPE + v transpose
            with (
                tc.tile_pool(name="proj_ps", bufs=NM, space="PSUM") as proj_ps,
                tc.tile_pool(name="vt_ps", bufs=2, space="PSUM") as vt_ps,
            ):
                for ng in range(ST // NGW):
                    c0 = ng * NGW
                    ptiles = [proj_ps.tile([128, NGW], f32, name=f"pp{ng}_{m}",
                                           tag="pp") for m in range(NM)]
                    for kt in range(KT):
                        xt = xpool.tile([128, NGW], bf16, name=f"x{ng}_{kt}",
                                        tag="xt")
                        nc.sync.dma_start(xt[:], xT_d[kt * 128:(kt + 1) * 128,
                                                      c0:c0 + NGW])
                        for m in range(NM):
                            for lo, hi in _segments(0, NGW):
                                nc.tensor.matmul(
                                    ptiles[m][:, lo:hi],
                                    wqkv_t[kt][:, m * 128:(m + 1) * 128],
                                    xt[:, lo:hi],
                                    start=(kt == 0), stop=(kt == KT - 1),
                                )

                    def rope_mats(src_sb, rows, tag, c0=c0, ng=ng):
                        """RoPE product terms for rows [0, rows) of src_sb."""
                        sh = scratch.tile([128, NGW], bf16, name=f"sh{ng}{tag}",
                                          tag="sh")
                        for r0 in range(0, rows, 64):
                            nc.vector.tensor_copy(sh[r0:r0 + 32, :],
                                                  src_sb[r0 + 32:r0 + 64, :])
                            nc.vector.tensor_copy(sh[r0 + 32:r0 + 64, :],
                                                  src_sb[r0:r0 + 32, :])
                        t1 = scratch.tile([128, NGW], bf16, name=f"t1{ng}{tag}",
                                          tag="t1")
                        nc.vector.tensor_mul(t1[0:rows, :], src_sb[0:rows, :],
                                             cos2[0:rows, c0:c0 + NGW])
                        t2 = scratch.tile([128, NGW], bf16, name=f"t2{ng}{tag}",
                                          tag="t2")
                        nc.vector.tensor_mul(t2[0:rows, :], sh[0:rows, :],
                                             sinadj[0:rows, c0:c0 + NGW])
                        return t1, t2

                    for m in range(NP):
                        qb = scratch.tile([128, NGW], bf16, name=f"qb{ng}{m}",
                                          tag="qb")
                        nc.scalar.copy(qb[:], ptiles[m][:])
                        t1, t2 = rope_mats(qb, 128, f"q{m}")
                        nc.vector.tensor_add(q2[m][:, c0:c0 + NGW], t1[:], t2[:])

                    kvb = scratch.tile([128, NGW], bf16, name=f"kvb{ng}", tag="qb")
                    nc.scalar.copy(kvb[0:64, :], ptiles[NM - 1][0:64, :])
                    t1, t2 = rope_mats(kvb, 64, "k")
                    nc.vector.tensor_add(k2[0:64, c0:c0 + NGW],
                                         t1[0:64, :], t2[0:64, :])
                    nc.vector.tensor_add(k2[64:128, c0:c0 + NGW],
                                         t1[0:64, :], t2[0:64, :])
                    # v rows [64:128) of psum -> base-0 tile -> token-major vaug
                    vb = scratch.tile([64, NGW], bf16, name=f"vb{ng}", tag="vb")
                    nc.vector.tensor_copy(vb[:], ptiles[NM - 1][64:128, :])
                    for j in range(NGW // 128):
                        tb = ng * (NGW // 128) + j
                        vt = vt_ps.tile([128, D], bf16, name=f"vt{tb}", tag="vt")
                        nc.tensor.transpose(
                            vt[:], vb[:, j * 128:(j + 1) * 128], ident[:])
                        nc.vector.tensor_copy(vaug[tb][:, 0:D], vt[:])
                        nc.vector.memset(vaug[tb][:, D:D + 1], 1.0)

            # ---------------- stage 2+3: attention (pair-packed) + out proj
            NQC = Sb // QCW
            NTT = QCW // 128        # token tiles per q-chunk
            hsegs = _segments(0, H)
            with (
                tc.tile_pool(name="sc_ps", bufs=2, space="PSUM") as sc_ps,
                tc.tile_pool(name="acc_ps", bufs=1, space="PSUM") as acc_ps,
                tc.tile_pool(name="o_ps", bufs=2, space="PSUM") as o_ps,
            ):
                for b in range(B):
                    b0 = b * Sb
                    for qc in range(NQC):
                        q0 = b0 + qc * QCW
                        nkt = (qc + 1) * (QCW // 128)
                        for p in range(NP):
                            accs = [acc_ps.tile([D + 1, QCW], f32,
                                                name=f"acc{b}{p}{qc}{h}",
                                                tag=f"acc{h}")
                                    for h in range(2)]
                            for kt in range(nkt):
                                r = kt * 128 - qc * QCW
                                w0 = max(0, r)
                                sc = sc_ps.tile([128, 2 * QCW], f32,
                                                name=f"sc{b}{p}{qc}{kt}",
                                                tag="sc")
                                kcols = slice(b0 + kt * 128, b0 + (kt + 1) * 128)
                                for h in range(2):
                                    hr = h * 64
                                    nc.tensor.matmul(
                                        sc[:, h * QCW + w0:(h + 1) * QCW],
                                        k2[hr:hr + 64, kcols],
                                        q2[p][hr:hr + 64, q0 + w0:q0 + QCW],
                                        start=True, stop=True)
                                et = etp.tile([128, 2 * QCW], bf16,
                                              name=f"et{b}{p}{qc}{kt}", tag="et")
                                sc3 = sc[:].rearrange("p (h w) -> p h w", h=2)
                                et3 = et[:].rearrange("p (h w) -> p h w", h=2)
                                nc.scalar.activation(et3[:, :, w0:QCW],
                                                     sc3[:, :, w0:QCW],
                                                     AF.Exp, scale=SCALE)
                                if r >= 0:
                                    for h in range(2):
                                        o = h * QCW + r
                                        nc.vector.tensor_mul(
                                            et[:, o:o + 128],
                                            et[:, o:o + 128], trimask[:])
                                va = vaug[(b0 // 128) + kt]
                                for h in range(2):
                                    nc.tensor.matmul(
                                        accs[h][:, w0:QCW],
                                        va[:, 0:D + 1],
                                        et[:, h * QCW + w0:(h + 1) * QCW],
                                        start=(kt == 0), stop=(kt == nkt - 1),
                                        skip_group_check=True)
                            for h in range(2):
                                acc = accs[h]
                                recip = npool.tile([1, QCW], f32,
                                                   name=f"rc{b}{p}{qc}{h}",
                                                   tag="recip")
                                nc.vector.reciprocal(recip[:],
                                                     acc[D:D + 1, :])
                                reciph = npool.tile([1, QCW], f16,
                                                    name=f"rh{b}{p}{qc}{h}",
                                                    tag="reciph")
                                nc.vector.tensor_copy(reciph[:], recip[:])
                                rb = sc_ps.tile([D, QCW], f32,
                                                name=f"rb{b}{p}{qc}{h}", tag="sc")
                                for lo, hi in _segments(0, QCW):
                                    nc.tensor.matmul(rb[:, lo:hi], ones64[:],
                                                     reciph[:, lo:hi],
                                                     start=True, stop=True)
                                rbs = npool.tile([D, QCW], f32,
                                                 name=f"rbs{b}{p}{qc}{h}",
                                                 tag="rbs")
                                nc.scalar.copy(rbs[:], rb[:])
                                nc.vector.tensor_mul(
                                    attnT[p][h * 64:h * 64 + 64, q0:q0 + QCW],
                                    acc[0:D, :], rbs[:])
                        # ---- out projection for this q-chunk's tokens
                        for tl in range(NTT):
                            t0 = q0 + tl * 128
                            ob = obuf.tile([128, H], bf16, name=f"ob{b}{qc}{tl}",
                                           tag="ob")
                            for hc, (lo, hi) in enumerate(hsegs):
                                ot = o_ps.tile([128, hi - lo], f32,
                                               name=f"ot{b}{qc}{tl}{hc}",
                                               tag="ot")
                                for ki in range(NP):
                                    nc.tensor.matmul(
                                        ot[:],
                                        attnT[ki][:, t0:t0 + 128],
                                        wo_t[ki][:, lo:hi],
                                        start=(ki == 0), stop=(ki == NP - 1))
                                nc.vector.tensor_copy(ob[:, lo:hi], ot[:])
                            nc.sync.dma_start(out_d[t0:t0 + 128, :], ob[:])

    nc.finalize()
    return nc


_CACHE = {}


def _get_nc(key, **kw):
    if key not in _CACHE:
        _CACHE[key] = build(**kw)
    return _CACHE[key]


def _prep_inputs(x, cos, sin, wq, wk, wv, wo):
    """Host-side sharding/layout prep. Returns list of 8 per-core in_maps."""
    Bx, S, H = x.shape
    bf = ml_dtypes.bfloat16
    x2d = x.reshape(Bx * S, H)
    xT = np.ascontiguousarray(x2d.T).astype(bf)

    cosT = np.concatenate([cos[b].T for b in range(Bx)], axis=1)   # [64, B*S]
    sinT = np.concatenate([sin[b].T for b in range(Bx)], axis=1)
    cos2 = np.tile(cosT, (2, 1)).astype(bf)
    sadj64 = np.concatenate([-sinT[0:32], sinT[32:64]], axis=0)
    sinadj = np.tile(sadj64, (2, 1)).astype(bf)

    in_maps = []
    for c in range(8):
        wq_c = wq[c * 256:(c + 1) * 256]          # (256, H)
        wk_c = wk[c * 64:(c + 1) * 64]            # (64, H)
        wv_c = wv[c * 64:(c + 1) * 64]
        wqkvT = np.concatenate([wq_c.T, wk_c.T, wv_c.T], axis=1).astype(bf)
        woT = np.ascontiguousarray(wo[:, c * 256:(c + 1) * 256].T).astype(bf)
        in_maps.append({
            "xT": xT, "cos2": cos2, "sinadj": sinadj,
            "wqkvT": np.ascontiguousarray(wqkvT),
            "woT": woT,
        })
    return in_maps


LAST_RESULTS = None


def kernel(x, cos, sin, mask, wq, wk, wv, wo):
    global LAST_RESULTS
    from concourse.bass_utils import run_bass_kernel_spmd

    x = np.asarray(x, dtype=np.float32)
    cos = np.asarray(cos, dtype=np.float32)
    sin = np.asarray(sin, dtype=np.float32)
    wq = np.asarray(wq, dtype=np.float32)
    wk = np.asarray(wk, dtype=np.float32)
    wv = np.asarray(wv, dtype=np.float32)
    wo = np.asarray(wo, dtype=np.float32)

    nc = _get_nc("full")
    in_maps = _prep_inputs(x, cos, sin, wq, wk, wv, wo)
    LAST_RESULTS = run_bass_kernel_spmd(nc, in_maps, core_ids=list(range(8)))
    Bx, S, H = x.shape
    out = np.zeros((Bx * S, H), dtype=np.float32)
    for r in LAST_RESULTS.results:
        out += r["out"].astype(np.float32)
    return out.reshape(Bx, S, H)



# revision 19
# speedup vs baseline: 1.4427x; 1.4427x over previous
"""Llama attention (B=2, S=2048, H=2048, NH=32, NKV=8, D=64) on 8 trn2 cores.

Sharding: tensor-parallel over heads. Core c owns q-heads [4c, 4c+4) and
kv-head c (GQA groups stay aligned). Each core computes its partial
out_c = attn_c @ wo[:, 256c:256c+256].T over the full batch/sequence in
bf16; the host sums the 8 partials in f32.

Device layout notes:
  - projections computed "feature-major": q/k [d, tok] via stationary
    weight tiles streaming xT; v is PE-transposed to token-major and
    augmented with a ones column so the AV matmul also produces softmax
    denominators.
  - q heads are stored pair-stacked ([head 2p | head 2p+1] on partitions)
    and k is duplicated into both partition halves, so the two scoresT
    matmuls of a pair run concurrently on disjoint PE row-groups (K=64
    each). One 3D-AP exp covers both heads.
  - causal: partial-width matmuls + a triangular mask multiply on the
    diagonal 128x128 block of each head.
  - the schedule keeps the PE warm (HAM K=8/8) and breaks cross-engine
    round trips: a warmup matmul burst covers initial DMA latency; the
    attention kt-loop emits score(kt+1) before AV(kt) so the ACT exp
    stream never waits on the PE (and vice versa); v-transposes of
    group g run inside group g+1's matmul stream; the out projection of
    q-chunk i is emitted after chunk i+1's score/AV slots so the
    softmax-normalization chain (fast DVE reciprocal -> f16 bcast
    matmul -> DVE multiply) never stalls the PE.
"""

import sys

if "/opt/trn_rl_repo" not in sys.path:
    sys.path.insert(0, "/opt/trn_rl_repo")

import numpy as np
import ml_dtypes

import concourse.bass as bass
import concourse.mybir as mybir
import concourse.tile as tile
from concourse import bacc
from concourse.masks import make_identity

bf16 = mybir.dt.bfloat16
f16 = mybir.dt.float16
f32 = mybir.dt.float32
AF = mybir.ActivationFunctionType

B = 2
D = 64
QH = 4                      # q heads per core
SCALE = D ** -0.5
VW = D + 2                  # vaug stride (64 v dims + ones col + pad)


def _segments(lo, hi, step=512):
    """Split [lo, hi) at multiples of `step` (matmul one-psum-bank limit)."""
    out = []
    while lo < hi:
        nxt = min(hi, (lo // step + 1) * step)
        out.append((lo, nxt))
        lo = nxt
    return out


def build(Sb=2048, H=2048, NGW=1024, QCW=512):
    """Sb: tokens per batch; H: model dim; NGW: stage-1 token group width;
    QCW: per-head q-chunk width in stage 2 (<= 512)."""
    assert QCW <= 512
    ST = B * Sb             # total tokens
    KT = H // 128           # contraction tiles for projections
    DQ = QH * D             # 256
    NP = QH // 2            # head pairs per core
    SEG = NGW // 512        # 512-wide psum segments per group
    TPG = NGW // 128        # token tiles per group

    nc = bacc.Bacc(trn_type="TRN2")
    xT_d = nc.dram_tensor("xT", [H, ST], bf16, kind="ExternalInput")
    wqkv_d = nc.dram_tensor("wqkvT", [H, DQ + 2 * D], bf16, kind="ExternalInput")
    wo_d = nc.dram_tensor("woT", [DQ, H], bf16, kind="ExternalInput")
    cos2_d = nc.dram_tensor("cos2", [128, ST], bf16, kind="ExternalInput")
    sinadj_d = nc.dram_tensor("sinadj", [128, ST], bf16, kind="ExternalInput")
    out_d = nc.dram_tensor("out", [ST, H], bf16, kind="ExternalOutput")
    warm_d = nc.dram_tensor("warmscr", [1, 64], f32, kind="Internal")

    with tile.TileContext(nc) as tc:
        with (
            tc.tile_pool(name="consts", bufs=1) as consts,
            tc.tile_pool(name="resident", bufs=1) as res,
            tc.tile_pool(name="xpool", bufs=4) as xpool,
            tc.tile_pool(name="scratch", bufs=3) as scratch,
            tc.tile_pool(name="etp", bufs=6) as etp,
            tc.tile_pool(name="npool", bufs=3) as npool,
            tc.tile_pool(name="avsp", bufs=4) as avsp,
            tc.tile_pool(name="obuf", bufs=3) as obuf,
        ):
            ident = consts.tile([D, D], bf16, name="ident")
            make_identity(nc, ident)
            ones64 = consts.tile([1, D], f16, name="ones64")
            nc.vector.memset(ones64[:], 1.0)
            dummy = consts.tile([128, 512], bf16, name="dummy")
            nc.vector.memset(dummy[:], 0.0)
            # block-ones for the denominator broadcast: a K=33 matmul with
            # lhsT wbig[b:b+33] maps rech4 row b -> psum rows 0:64 and row
            # b+32 -> psum rows 64:128 (rows {0,32,64,96} keep every DVE
            # copy and matmul base 32-aligned)
            wbig = consts.tile([97, 128], f16, name="wbig")
            nc.vector.memset(wbig[:], 0.0)
            for rw in (0, 64):
                nc.vector.memset(wbig[rw:rw + 1, 0:64], 1.0)
                nc.vector.memset(wbig[rw + 32:rw + 33, 64:128], 1.0)
            trimask = consts.tile([128, 128], bf16, name="trimask")
            nc.vector.memset(trimask[:], 1.0)
            nc.gpsimd.affine_select(
                out=trimask[:], in_=trimask[:],
                compare_op=mybir.AluOpType.is_ge, fill=0.0,
                base=0, pattern=[[1, 128]], channel_multiplier=-1,
            )

            wqkv_t = []
            for kt in range(KT):
                w = res.tile([128, DQ + 2 * D], bf16, name=f"wqkv{kt}")
                nc.sync.dma_start(w[:], wqkv_d[kt * 128:(kt + 1) * 128, :])
                wqkv_t.append(w)

            cos2 = res.tile([128, ST], bf16, name="cos2")
            nc.sync.dma_start(cos2[:], cos2_d[:])
            sinadj = res.tile([128, ST], bf16, name="sinadj")
            nc.sync.dma_start(sinadj[:], sinadj_d[:])

            # pair-stacked q, duplicated k, pair-stacked attention output
            q2 = [res.tile([128, ST], bf16, name=f"q2_{p}") for p in range(NP)]
            k2 = res.tile([128, ST], bf16, name="k2")
            attnT = [res.tile([128, ST], bf16, name=f"attnT{p}") for p in range(NP)]
            # token-major v, one strip of [64 dims | ones | pad] per 128-token
            # tile, all in one resident tensor (ones come from the memset)
            vaugAll = res.tile([128, (ST // 128) * VW], bf16, name="vaugAll")
            nc.vector.memset(vaugAll[:], 1.0)

            # ---------------- stage 1: QKV projection + RoPE + v transpose
            NG = ST // NGW

            def emit_transposes(pv, pool):
                g, vb = pv
                vt = pool.tile([128, TPG * D], bf16, name=f"vt{g}", tag="vt")
                for j in range(TPG):
                    nc.tensor.transpose(
                        vt[:, j * D:(j + 1) * D],
                        vb[:, j * 128:(j + 1) * 128], ident[:])
                dst = vaugAll[:, g * TPG * VW:(g + 1) * TPG * VW]
                dst3 = dst.rearrange("p (t c) -> p t c", t=TPG)
                vt3 = vt[:].rearrange("p (t c) -> p t c", t=TPG)
                nc.vector.tensor_copy(dst3[:, :, 0:D], vt3[:])

            with (
                tc.tile_pool(name="proj_ps", bufs=6, space="PSUM") as proj_ps,
                tc.tile_pool(name="vt_ps", bufs=2, space="PSUM") as vt_ps,
            ):
                # PE warmup burst: covers initial DMA latency and trips the
                # HAM activity window to K=8/8 before real matmuls start.
                wm = proj_ps.tile([128, 512], f32, name="warm", tag="pp")
                for i in range(20):
                    nc.tensor.matmul(wm[:], dummy[:, 0:128], dummy[:],
                                     start=True, stop=True)
                wmsb = consts.tile([1, D], f32, name="wmsb")
                nc.vector.tensor_copy(wmsb[:], wm[0:1, 0:D])
                nc.sync.dma_start(warm_d[:], wmsb[:])

                def rope_mats(src_sb, rows, tag, c0):
                    """RoPE product terms for rows [0, rows) of src_sb."""
                    sh = scratch.tile([128, NGW], bf16, name=f"sh{tag}",
                                      tag="sh")
                    for r0 in range(0, rows, 64):
                        nc.vector.tensor_copy(sh[r0:r0 + 32, :],
                                              src_sb[r0 + 32:r0 + 64, :])
                        nc.vector.tensor_copy(sh[r0 + 32:r0 + 64, :],
                                              src_sb[r0:r0 + 32, :])
                    t1 = scratch.tile([128, NGW], bf16, name=f"t1{tag}",
                                      tag="t1")
                    nc.vector.tensor_mul(t1[0:rows, :], src_sb[0:rows, :],
                                         cos2[0:rows, c0:c0 + NGW])
                    t2 = scratch.tile([128, NGW], bf16, name=f"t2{tag}",
                                      tag="t2")
                    nc.vector.tensor_mul(t2[0:rows, :], sh[0:rows, :],
                                         sinadj[0:rows, c0:c0 + NGW])
                    return t1, t2

                pend_v = None
                for g in range(NG):
                    c0 = g * NGW
                    pt = [[proj_ps.tile([128, 512], f32,
                                        name=f"pp{g}_{m}_{s}", tag="pp")
                           for s in range(SEG)] for m in range(NP + 1)]
                    for kt in range(KT):
                        xt = xpool.tile([128, NGW], bf16, name=f"x{g}_{kt}",
                                        tag="xt")
                        nc.sync.dma_start(xt[:], xT_d[kt * 128:(kt + 1) * 128,
                                                      c0:c0 + NGW])
                        for m in range(NP + 1):
                            for s in range(SEG):
                                nc.tensor.matmul(
                                    pt[m][s][:],
                                    wqkv_t[kt][:, m * 128:(m + 1) * 128],
                                    xt[:, s * 512:(s + 1) * 512],
                                    start=(kt == 0), stop=(kt == KT - 1),
                                )
                        if kt == 1 and pend_v is not None:
                            # previous group's v transposes: PE work that
                            # bridges the psum-ring handover
                            emit_transposes(pend_v, vt_ps)
                            pend_v = None

                    # v rows [64:128) -> token-major staging (ACT frees psum)
                    vb = scratch.tile([64, NGW], bf16, name=f"vb{g}", tag="vb")
                    for s in range(SEG):
                        nc.scalar.copy(vb[:, s * 512:(s + 1) * 512],
                                       pt[NP][s][64:128, :])
                    pend_v = (g, vb)

                    for m in range(NP):
                        qb = scratch.tile([128, NGW], bf16, name=f"qb{g}{m}",
                                          tag="qb")
                        for s in range(SEG):
                            nc.scalar.copy(qb[:, s * 512:(s + 1) * 512],
                                           pt[m][s][:])
                        t1, t2 = rope_mats(qb, 128, f"{g}q{m}", c0)
                        nc.vector.tensor_add(q2[m][:, c0:c0 + NGW], t1[:], t2[:])

                    kvb = scratch.tile([128, NGW], bf16, name=f"kvb{g}",
                                       tag="qb")
                    for s in range(SEG):
                        nc.scalar.copy(kvb[0:64, s * 512:(s + 1) * 512],
                                       pt[NP][s][0:64, :])
                    t1, t2 = rope_mats(kvb, 64, f"{g}k", c0)
                    nc.vector.tensor_add(k2[0:64, c0:c0 + NGW],
                                         t1[0:64, :], t2[0:64, :])
                    nc.vector.tensor_add(k2[64:128, c0:c0 + NGW],
                                         t1[0:64, :], t2[0:64, :])

                last_v = pend_v

            wo_t = []
            for ki in range(DQ // 128):
                w = res.tile([128, H], bf16, name=f"wo{ki}")
                nc.sync.dma_start(w[:], wo_d[ki * 128:(ki + 1) * 128, :])
                wo_t.append(w)

            # last group's v transposes bridge the stage-1 RoPE tail
            with tc.tile_pool(name="vt2_ps", bufs=1, space="PSUM") as vt2_ps:
                emit_transposes(last_v, vt2_ps)

            # ---------------- stage 2+3: attention (pair-packed) + out proj
            NQC = Sb // QCW
            NTT = QCW // 128        # token tiles per q-chunk
            hsegs = _segments(0, H)
            with (
                tc.tile_pool(name="sc_ps", bufs=2, space="PSUM") as sc_ps,
                tc.tile_pool(name="acc_ps", bufs=1, space="PSUM") as acc_ps,
                tc.tile_pool(name="po_ps", bufs=2, space="PSUM") as po_ps,
            ):
                def emit_norm_pair(jobs2, rech4, base):
                    rb2 = po_ps.tile([128, QCW], f32,
                                     name=f"rb{base}{jobs2[0][2]}", tag="po")
                    nc.tensor.matmul(rb2[:], wbig[base:base + 33, :],
                                     rech4[base:base + 33, :],
                                     start=True, stop=True)
                    for i, (p, h, q0, avs) in enumerate(jobs2):
                        nc.vector.tensor_mul(
                            attnT[p][h * 64:h * 64 + 64, q0:q0 + QCW],
                            avs[:], rb2[i * 64:(i + 1) * 64, :])

                def emit_outproj(bqc, tls):
                    bb, qq = bqc
                    for tl in tls:
                        t0 = bb * Sb + qq * QCW + tl * 128
                        ob = obuf.tile([128, H], bf16, name=f"ob{t0}",
                                       tag="ob")
                        for hc, (lo, hi) in enumerate(hsegs):
                            ot = po_ps.tile([128, hi - lo], f32,
                                            name=f"ot{t0}{hc}", tag="po")
                            for ki in range(NP):
                                nc.tensor.matmul(
                                    ot[:],
                                    attnT[ki][:, t0:t0 + 128],
                                    wo_t[ki][:, lo:hi],
                                    start=(ki == 0), stop=(ki == NP - 1))
                            if hc == len(hsegs) - 1:
                                nc.scalar.copy(ob[:, lo:hi], ot[:])
                            else:
                                nc.vector.tensor_copy(ob[:, lo:hi], ot[:])
                        nc.sync.dma_start(out_d[t0:t0 + 128, :], ob[:])

                prev = None
                for b in range(B):
                    b0 = b * Sb
                    for qc in range(NQC):
                        q0 = b0 + qc * QCW
                        nkt = (qc + 1) * (QCW // 128)
                        jobs = []
                        sums4 = npool.tile([97, QCW], f32,
                                           name=f"s4{b}{qc}", tag="s4")
                        if b == 0 and qc < 3:
                            # first pass through the ring: ensure the unused
                            # partitions hold finite values (0*NaN poisons
                            # the K=33 broadcast matmul otherwise)
                            nc.vector.memset(sums4[:], 1.0)
                        accs = {p: [acc_ps.tile([D + 1, QCW], f32,
                                                name=f"acc{b}{p}{qc}{h}",
                                                tag=f"acc{h}")
                                    for h in range(2)]
                                for p in range(NP)}

                        def emit_score(p, kt):
                            r = kt * 128 - qc * QCW
                            w0 = max(0, r)
                            sc = sc_ps.tile([128, 2 * QCW], f32,
                                            name=f"sc{b}{p}{qc}{kt}",
                                            tag="sc")
                            kcols = slice(b0 + kt * 128, b0 + (kt + 1) * 128)
                            for h in range(2):
                                hr = h * 64
                                nc.tensor.matmul(
                                    sc[:, h * QCW + w0:(h + 1) * QCW],
                                    k2[hr:hr + 64, kcols],
                                    q2[p][hr:hr + 64, q0 + w0:q0 + QCW],
                                    start=True, stop=True)
                            et = etp.tile([128, 2 * QCW], bf16,
                                          name=f"et{b}{p}{qc}{kt}", tag="et")
                            sc3 = sc[:].rearrange("p (h w) -> p h w", h=2)
                            et3 = et[:].rearrange("p (h w) -> p h w", h=2)
                            nc.scalar.activation(et3[:, :, w0:QCW],
                                                 sc3[:, :, w0:QCW],
                                                 AF.Exp, scale=SCALE)
                            if r >= 0:
                                for h in range(2):
                                    o = h * QCW + r
                                    nc.vector.tensor_mul(
                                        et[:, o:o + 128],
                                        et[:, o:o + 128], trimask[:])
                            return (p, kt, w0, et)

                        def emit_av(slot):
                            p, kt, w0, et = slot
                            va = vaugAll[:, ((b0 // 128) + kt) * VW:
                                         ((b0 // 128) + kt) * VW + D + 1]
                            for h in range(2):
                                nc.tensor.matmul(
                                    accs[p][h][:, w0:QCW],
                                    va[:],
                                    et[:, h * QCW + w0:(h + 1) * QCW],
                                    start=(kt == 0), stop=(kt == nkt - 1),
                                    skip_group_check=True)
                            if kt == nkt - 1:
                                # copy unnormalized AV + sums out of psum
                                # immediately (frees the acc bank); sums
                                # rows collect into one [4, QCW] tile so a
                                # single partition-parallel reciprocal
                                # covers the whole chunk
                                for h in range(2):
                                    row = (p * 2 + h) * 32
                                    avs = avsp.tile([D, QCW], f32,
                                                    name=f"av{b}{p}{qc}{h}",
                                                    tag="avs")
                                    nc.vector.tensor_copy(avs[:],
                                                          accs[p][h][0:D, :])
                                    nc.vector.tensor_copy(
                                        sums4[row:row + 1, :],
                                        accs[p][h][D:D + 1, :])
                                    jobs.append((p, h, q0, avs))

                        # one-slot software pipeline: score(i+1) is emitted
                        # before AV(i) so the ACT exp stream and the PE
                        # never round-trip wait on each other
                        pend = None
                        for p in range(NP):
                            for kt in range(nkt):
                                slot = emit_score(p, kt)
                                if pend is not None:
                                    emit_av(pend)
                                pend = slot
                        emit_av(pend)

                        # one reciprocal + f16 cast for the chunk's four
                        # denominator rows; bcast matmuls interleave with
                        # the previous chunk's out projection so the PE
                        # never waits on the DVE reciprocal chain
                        rec4 = npool.tile([97, QCW], f32,
                                          name=f"rec{b}{qc}", tag="rec4")
                        nc.vector.reciprocal(rec4[:], sums4[:])
                        rech4 = npool.tile([97, QCW], f16,
                                           name=f"rh{b}{qc}", tag="rech4")
                        nc.vector.tensor_copy(rech4[:], rec4[:])
                        if prev is not None:
                            emit_outproj(prev, range(0, NTT // 2))
                        emit_norm_pair(jobs[0:2], rech4, 0)
                        if prev is not None:
                            emit_outproj(prev, range(NTT // 2, NTT))
                        emit_norm_pair(jobs[2:4], rech4, 64)
                        prev = (b, qc)
                emit_outproj(prev, range(NTT))

    nc.finalize()
    return nc


_CACHE = {}


def _get_nc(key, **kw):
    if key not in _CACHE:
        _CACHE[key] = build(**kw)
    return _CACHE[key]


def _prep_inputs(x, cos, sin, wq, wk, wv, wo):
    """Host-side sharding/layout prep. Returns list of 8 per-core in_maps."""
    Bx, S, H = x.shape
    bf = ml_dtypes.bfloat16
    x2d = x.reshape(Bx * S, H)
    xT = np.ascontiguousarray(x2d.T).astype(bf)

    cosT = np.concatenate([cos[b].T for b in range(Bx)], axis=1)   # [64, B*S]
    sinT = np.concatenate([sin[b].T for b in range(Bx)], axis=1)
    cos2 = np.tile(cosT, (2, 1)).astype(bf)
    sadj64 = np.concatenate([-sinT[0:32], sinT[32:64]], axis=0)
    sinadj = np.tile(sadj64, (2, 1)).astype(bf)

    in_maps = []
    for c in range(8):
        wq_c = wq[c * 256:(c + 1) * 256]          # (256, H)
        wk_c = wk[c * 64:(c + 1) * 64]            # (64, H)
        wv_c = wv[c * 64:(c + 1) * 64]
        wqkvT = np.concatenate([wq_c.T, wk_c.T, wv_c.T], axis=1).astype(bf)
        woT = np.ascontiguousarray(wo[:, c * 256:(c + 1) * 256].T).astype(bf)
        in_maps.append({
            "xT": xT, "cos2": cos2, "sinadj": sinadj,
            "wqkvT": np.ascontiguousarray(wqkvT),
            "woT": woT,
        })
    return in_maps


LAST_RESULTS = None


def kernel(x, cos, sin, mask, wq, wk, wv, wo):
    global LAST_RESULTS
    from concourse.bass_utils import run_bass_kernel_spmd

    x = np.asarray(x, dtype=np.float32)
    cos = np.asarray(cos, dtype=np.float32)
    sin = np.asarray(sin, dtype=np.float32)
    wq = np.asarray(wq, dtype=np.float32)
    wk = np.asarray(wk, dtype=np.float32)
    wv = np.asarray(wv, dtype=np.float32)
    wo = np.asarray(wo, dtype=np.float32)

    nc = _get_nc("full")
    in_maps = _prep_inputs(x, cos, sin, wq, wk, wv, wo)
    LAST_RESULTS = run_bass_kernel_spmd(nc, in_maps, core_ids=list(range(8)))
    Bx, S, H = x.shape
    out = np.zeros((Bx * S, H), dtype=np.float32)
    for r in LAST_RESULTS.results:
        out += r["out"].astype(np.float32)
    return out.reshape(Bx, S, H)
